# revision 26
# baseline (speedup 1.0000x reference)
"""HQQLinearLoRA TRN2 kernel: out = x @ W + (x @ A) @ B * 1.0 + bias.

Sharding: data-parallel over tokens (B*S) across 8 NeuronCores; W/bias/lora
replicated. Per core: [M_CORE, D] @ [D, D] with LoRA rank-16 + bias folded
into one extra K=17 accumulation matmul per output tile.

PE runs float32r (1 cycle/row when the moving dim >= 256). Every f32r
matmul operand must be last-produced by a rounding instruction (BIR
verifier rule), so W tiles are DMA'd as f32 and rounded by DVE copies;
x is transposed on PE in f32 and the PSUM->SBUF copyback rounds to f32r.

Structure (PE is the bottleneck; everything else is shaped to keep it fed):
- m_blocks=[8,8]: W is streamed twice (201MB total DMA/core vs 335MB at
  mb=4).
- The x-transpose runs fused inside each block's FIRST n-tile pass (no
  standalone transpose phase => no PE bubble at block boundaries). That
  pass is split into two half-passes of 4 m-tiles so PSUM fits:
  4 main accumulators + 1 p1t (LoRA xT@A) + 3 rotating transpose banks.
- Transposes run one k-tile ahead of the matmuls that consume them so the
  DVE copyback hides under the previous k-tile's matmuls.
- LoRA + bias are one K=17 matmul: p1t row 16 is ones (whole tile pre-
  filled with rounded 1.0 since engines can't write at partition base 16),
  b17 = [lora_B; bias] staged f32 then rounded.
- The last k-iteration interleaves [matmul, lora+stop] per m-tile so PSUM
  drains start ~1.7us earlier and the next n-tile never waits on a drain.
- Constant staging copies are spread across the first k-loop so the DVE
  queue is never busy with them when PE needs a wr tile or copyback.
- Queues: W tiles on SP HWDGE, x chunks on gpsimd SWDGE, out stores on
  Activation HWDGE.
"""
import numpy as np
from contextlib import ExitStack

import concourse.bass as bass
import concourse.tile as tile
import concourse.mybir as mybir
from concourse import bacc
from concourse.bass_utils import run_bass_kernel_spmd
from concourse.masks import make_identity

P = 128
NCORES = 8

# full problem dims (hardcoded per task contract)
B_DIM, S_DIM, D_DIM, R_DIM = 4, 4096, 4096, 16


def build_nc(m_core, d, r, m_blocks, n_tile=512, kg=4,
             xs_bufs=8, ws_bufs=5, wr_bufs=3, ot_bufs=4):
    """One-core program; same program runs SPMD on all cores."""
    KT = d // P
    NT = d // n_tile
    R17 = r + 1
    f32 = mybir.dt.float32
    f32r = mybir.dt.float32r

    nc = bacc.Bacc(target_bir_lowering=False)
    x = nc.declare_dram_parameter("x", [m_core, d], f32, isOutput=False)
    W = nc.declare_dram_parameter("W", [d, d], f32, isOutput=False)
    bias = nc.declare_dram_parameter("bias", [d], f32, isOutput=False)
    lora_A = nc.declare_dram_parameter("lora_A", [d, r], f32, isOutput=False)
    lora_B = nc.declare_dram_parameter("lora_B", [r, d], f32, isOutput=False)
    out = nc.declare_dram_parameter("out", [m_core, d], f32, isOutput=True)

    with tile.TileContext(nc) as tc, ExitStack() as ctx:
        const = ctx.enter_context(tc.tile_pool(name="const", bufs=1))
        stg = ctx.enter_context(tc.tile_pool(name="stg", bufs=1))
        xsp = ctx.enter_context(tc.tile_pool(name="xsp", bufs=xs_bufs))
        wstage = ctx.enter_context(tc.tile_pool(name="wstage", bufs=ws_bufs))
        wrpool = ctx.enter_context(tc.tile_pool(name="wrpool", bufs=wr_bufs))
        otp = ctx.enter_context(tc.tile_pool(name="otp", bufs=ot_bufs))
        xtbp = ctx.enter_context(tc.tile_pool(name="xtbp", bufs=1))
        ps_main = ctx.enter_context(
            tc.tile_pool(name="ps_main", bufs=4, space="PSUM"))
        ps_p1t = ctx.enter_context(
            tc.tile_pool(name="ps_p1t", bufs=1, space="PSUM"))
        ps_aux = ctx.enter_context(
            tc.tile_pool(name="ps_aux", bufs=3, space="PSUM"))

        # identity for PE transpose (f32 path)
        ident = const.tile([P, P], f32)
        make_identity(nc, ident)

        # lora_A as [P, KT, r] f32r (one DMA; partition p holds rows ki*P+p)
        a_st = stg.tile([P, KT, r], f32, name="stg")
        nc.scalar.dma_start(
            a_st[:], lora_A.rearrange("(k p) r -> p k r", p=P))
        a_r = const.tile([P, KT, r], f32r)
        # (a_r rounding copy is emitted inside the first k-loop)

        b17 = const.tile([R17, d], f32r)
        p1t = const.tile([R17, m_core], f32r)

        def stage_consts(step):
            # Emitted mid-k-loop of the first half-pass: keeps these DVE
            # copies out of the startup critical path.
            if step < 4:
                # b17 = [lora_B; bias]: [R17, d] f32r, staged in quarters
                qd = d // 4
                hsl = slice(step * qd, (step + 1) * qd)
                bst = stg.tile([R17, qd], f32, name="stg")
                nc.scalar.dma_start(bst[0:r, :], lora_B[:, hsl])
                nc.scalar.dma_start(bst[r:R17, :], bias[hsl].unsqueeze(0))
                nc.vector.tensor_copy(b17[:, hsl], bst[:])
            else:
                # p1t pre-fill with rounded 1.0: row 16 keeps it (ones for
                # the bias outer product); rows 0..16 get overwritten by
                # the per-half-pass chunk copies. Staged in halves.
                hm = m_core // 2
                hsl = slice((step - 4) * hm, (step - 3) * hm)
                p1st = stg.tile([R17, hm], f32, name="stg")
                nc.gpsimd.memset(p1st[:], 1.0)
                nc.vector.tensor_copy(p1t[:, hsl], p1st[:])

        # W tiles are emitted one consumer-step ahead (DMA + DVE rounding
        # copy), so at n-tile/block boundaries the next wr is already
        # rounded before the PSUM drain copies queue up on DVE.
        wsteps = []
        for mb in m_blocks:
            for hp in range(mb // 4):
                wsteps += [(0, ki) for ki in range(KT)]
            for ni in range(1, NT):
                wsteps += [(ni, ki) for ki in range(KT)]
        wq = []
        widx = [0]

        def emit_next_w():
            if widx[0] < len(wsteps):
                ni, ki = wsteps[widx[0]]
                widx[0] += 1
                nsl = slice(ni * n_tile, (ni + 1) * n_tile)
                ws = wstage.tile([P, n_tile], f32, name="ws")
                nc.sync.dma_start(ws[:], W[ki * P:(ki + 1) * P, nsl])
                wr = wrpool.tile([P, n_tile], f32r, name="wr")
                # alternate rounding copies between DVE and Act so neither
                # queue is the wr critical path during the fused pass
                if widx[0] % 2:
                    nc.vector.tensor_copy(wr[:], ws[:])
                else:
                    nc.scalar.copy(wr[:], ws[:])
                wq.append(wr)

        emit_next_w()

        out_dmas = []
        pend = [None]

        def stage_out(ps_tile, mt, nsl_):
            # PSUM -> SBUF, paired: two adjacent m-tiles share one
            # staging tile and one DRAM store (halves the Act SEQ
            # DMA-prep serialization at boundaries). First half on
            # DVE, second on Act so the drain burst fits both queues.
            if pend[0] is None:
                ot2 = otp.tile([P, 2, n_tile], f32, name="ot")
                pend[0] = (ot2, mt)
                nc.vector.tensor_copy(ot2[:, 0, :], ps_tile[:])
            else:
                ot2, lo = pend[0]
                pend[0] = None
                assert mt == lo + 1
                nc.scalar.copy(ot2[:, 1, :], ps_tile[:])
                while out_dmas:
                    emit_out_dma(*out_dmas.pop(0))
                out_dmas.append((lo, nsl_, ot2))

        odma_flip = [0]

        def emit_out_dma(lo, nsl_, ot2):
            # alternate queues so back-to-back stores transfer in parallel
            eng = nc.scalar if odma_flip[0] % 2 == 0 else nc.sync
            odma_flip[0] += 1
            eng.dma_start(
                out[lo * P:(lo + 2) * P, nsl_].rearrange(
                    "(j p) f -> p j f", p=P), ot2[:])

        def flush_out_dmas():
            while out_dmas:
                emit_out_dma(*out_dmas.pop(0))

        mt0 = 0
        first = True
        for mb in m_blocks:
            xtb = xtbp.tile([P, KT, mb * P], f32r, name="xtb")
            nhp = mb // 4
            for hp in range(nhp):
                # ---- fused first n-tile pass (ni=0): transpose + GEMM
                mis = [hp * 4 + i for i in range(4)]
                nsl = slice(0, n_tile)
                pss = [ps_main.tile([P, n_tile], f32, name="mm") for _ in mis]
                pp1 = ps_p1t.tile([r, n_tile], f32, name="mm")
                xs_tiles = {}

                def load_group(g, split=False):
                    for i, mi in enumerate(mis):
                        mt = mt0 + mi
                        xst = xsp.tile([P, kg * P], f32, name="xs")
                        # the very first group gates kernel startup: Pool's
                        # SWDGE descriptor gen is ~1us per DMA serial, so
                        # split it across the Pool and Act queues
                        eng = nc.scalar if split and i % 2 else nc.gpsimd
                        eng.dma_start(
                            xst[:],
                            x[mt * P:(mt + 1) * P, g * kg * P:(g + 1) * kg * P])
                        xs_tiles[(mi, g)] = xst

                def transpose_ki(ki):
                    g, lk = ki // kg, ki % kg
                    for mi in mis:
                        pst = ps_aux.tile([P, P], f32, name="mm")
                        nc.tensor.transpose(
                            pst[:],
                            xs_tiles[(mi, g)][:, lk * P:(lk + 1) * P],
                            ident[:])
                        # copyback rounds to f32r for the main matmuls
                        nc.vector.tensor_copy(
                            xtb[:, ki, mi * P:(mi + 1) * P], pst[:])

                def drain(j, mi):
                    # lora+bias accumulation, then drain
                    mt = mt0 + mi
                    nc.tensor.matmul(
                        pss[j][:], p1t[:, mt * P:(mt + 1) * P], b17[:, nsl],
                        start=False, stop=True)
                    stage_out(pss[j], mt, nsl)

                def pp1_mm(ki):
                    nc.tensor.matmul(
                        pp1[:], a_r[:, ki, :],
                        xtb[:, ki, hp * n_tile:(hp + 1) * n_tile],
                        start=(ki == 0), stop=(ki == KT - 1))

                load_group(0, split=first)
                transpose_ki(0)
                for ki in range(KT):
                    wr = wq.pop(0)
                    emit_next_w()
                    if ki % kg == 0 and ki + kg < KT:
                        load_group(ki // kg + 1)
                    if ki + 1 < KT:
                        transpose_ki(ki + 1)
                    last = ki == KT - 1
                    for j, mi in enumerate(mis):
                        nc.tensor.matmul(
                            pss[j][:], xtb[:, ki, mi * P:(mi + 1) * P],
                            wr[:], start=(ki == 0), stop=False)
                        if last:
                            drain(j, mi)
                    if first and ki == 0:
                        # a_r rounding copy off the startup critical path
                        # (first needed by pp1_mm(0) below)
                        nc.vector.tensor_copy(a_r[:], a_st[:])
                    # pp1 runs one k-tile behind the transposes but one AHEAD
                    # of this loop for ki>=1, so its stop lands in iteration
                    # KT-2 and the p1t rounding copy hides under the last
                    # main matmuls.
                    if ki == 0:
                        pp1_mm(0)
                        pp1_mm(1)
                    elif ki < KT - 1:
                        pp1_mm(ki + 1)
                        if ki == KT - 2:
                            nc.vector.tensor_copy(
                                p1t[0:r, (mt0 + hp * 4) * P:
                                    (mt0 + hp * 4) * P + n_tile],
                                pp1[:])
                    if first and 4 <= ki < 28 and ki % 4 == 0:
                        stage_consts(ki // 4 - 1)
                flush_out_dmas()
                first = False

            # ---- remaining n-tiles: plain GEMM with 8-wide PSUM
            for ni in range(1, NT):
                nsl = slice(ni * n_tile, (ni + 1) * n_tile)
                pss = []
                for mi in range(mb):
                    pool = (ps_main if mi < 4 else
                            ps_aux if mi < 7 else ps_p1t)
                    pss.append(pool.tile([P, n_tile], f32, name="mm"))
                # at the last n-tile of a block, drain the aux/p1t banks
                # first: the next block's transposes/pp1 need those slots
                # before the main banks
                mi_order = (list(range(4, mb)) + list(range(4))
                            if ni == NT - 1 else list(range(mb)))
                for ki in range(KT):
                    wr = wq.pop(0)
                    emit_next_w()
                    last = ki == KT - 1
                    for mi in (mi_order if last else range(mb)):
                        mt = mt0 + mi
                        nc.tensor.matmul(
                            pss[mi][:], xtb[:, ki, mi * P:(mi + 1) * P],
                            wr[:], start=(ki == 0), stop=False)
                        if last:
                            nc.tensor.matmul(
                                pss[mi][:], p1t[:, mt * P:(mt + 1) * P],
                                b17[:, nsl], start=False, stop=True)
                            stage_out(pss[mi], mt, nsl)
                flush_out_dmas()
            mt0 += mb
    nc.compile()
    return nc


_CACHE = {}


def _get_nc(key, *args, **kw):
    if key not in _CACHE:
        _CACHE[key] = build_nc(*args, **kw)
    return _CACHE[key]


def kernel(x, W, bias, lora_A, lora_B, _trace=False):
    Bb, S, D = x.shape
    R = lora_A.shape[1]
    M = Bb * S
    m_core = M // NCORES
    m_blocks = [8, 8]
    nc = _get_nc(("full", m_core, D, R), m_core, D, R, m_blocks)

    xf = np.ascontiguousarray(x.reshape(M, D), dtype=np.float32)
    W = np.ascontiguousarray(W, dtype=np.float32)
    bias = np.ascontiguousarray(bias, dtype=np.float32)
    lora_A = np.ascontiguousarray(lora_A, dtype=np.float32)
    lora_B = np.ascontiguousarray(lora_B, dtype=np.float32)

    in_maps = []
    for c in range(NCORES):
        in_maps.append({
            "x": xf[c * m_core:(c + 1) * m_core],
            "W": W, "bias": bias, "lora_A": lora_A, "lora_B": lora_B,
        })
    res = run_bass_kernel_spmd(nc, in_maps, list(range(NCORES)), trace=_trace)
    outs = [res.results[c]["out"] for c in range(NCORES)]
    full = np.concatenate(outs, axis=0).reshape(Bb, S, D).astype(x.dtype)
    if _trace:
        return full, res
    return full


# revision 37
# speedup vs baseline: 1.0040x; 1.0040x over previous
"""HQQLinearLoRA TRN2 kernel: out = x @ W + (x @ A) @ B * 1.0 + bias.

Sharding: data-parallel over tokens (B*S) across 8 NeuronCores; W/bias/lora
replicated. Per core: [M_CORE, D] @ [D, D] with LoRA rank-16 + bias folded
into one extra K=17 accumulation matmul per output tile.

PE runs float32r (1 cycle/row when the moving dim >= 256). Every f32r
matmul operand must be last-produced by a rounding instruction (BIR
verifier rule), so W tiles are DMA'd as f32 and rounded by DVE copies;
x is transposed on PE in f32 and the PSUM->SBUF copyback rounds to f32r.

Structure (PE is the bottleneck; everything else is shaped to keep it fed):
- m_blocks=[8,8]: W is streamed twice (201MB total DMA/core vs 335MB at
  mb=4).
- The x-transpose runs fused inside each block's FIRST n-tile pass (no
  standalone transpose phase => no PE bubble at block boundaries). That
  pass is split into two half-passes of 4 m-tiles so PSUM fits:
  4 main accumulators + 1 p1t (LoRA xT@A) + 3 rotating transpose banks.
- Transposes run one k-tile ahead of the matmuls that consume them so the
  DVE copyback hides under the previous k-tile's matmuls.
- LoRA + bias are one K=17 matmul: p1t row 16 is ones (whole tile pre-
  filled with rounded 1.0 since engines can't write at partition base 16),
  b17 = [lora_B; bias] staged f32 then rounded.
- The last k-iteration interleaves [matmul, lora+stop] per m-tile so PSUM
  drains start ~1.7us earlier and the next n-tile never waits on a drain.
- Constant staging copies are spread across the first k-loop so the DVE
  queue is never busy with them when PE needs a wr tile or copyback.
- Queues: W tiles on SP HWDGE, x chunks on gpsimd SWDGE, out stores on
  Activation HWDGE.
"""
import numpy as np
from contextlib import ExitStack

import concourse.bass as bass
import concourse.tile as tile
import concourse.mybir as mybir
from concourse import bacc
from concourse.bass_utils import run_bass_kernel_spmd
from concourse.masks import make_identity

P = 128
NCORES = 8

# full problem dims (hardcoded per task contract)
B_DIM, S_DIM, D_DIM, R_DIM = 4, 4096, 4096, 16


def build_nc(m_core, d, r, m_blocks, n_tile=512, kg=4,
             xs_bufs=8, ws_bufs=5, wr_bufs=3, ot_bufs=4):
    """One-core program; same program runs SPMD on all cores."""
    KT = d // P
    NT = d // n_tile
    R17 = r + 1
    f32 = mybir.dt.float32
    f32r = mybir.dt.float32r

    nc = bacc.Bacc(target_bir_lowering=False)
    x = nc.declare_dram_parameter("x", [m_core, d], f32, isOutput=False)
    W = nc.declare_dram_parameter("W", [d, d], f32, isOutput=False)
    bias = nc.declare_dram_parameter("bias", [d], f32, isOutput=False)
    lora_A = nc.declare_dram_parameter("lora_A", [d, r], f32, isOutput=False)
    lora_B = nc.declare_dram_parameter("lora_B", [r, d], f32, isOutput=False)
    out = nc.declare_dram_parameter("out", [m_core, d], f32, isOutput=True)

    with tile.TileContext(nc) as tc, ExitStack() as ctx:
        const = ctx.enter_context(tc.tile_pool(name="const", bufs=1))
        stg = ctx.enter_context(tc.tile_pool(name="stg", bufs=1))
        xsp = ctx.enter_context(tc.tile_pool(name="xsp", bufs=xs_bufs))
        wstage = ctx.enter_context(tc.tile_pool(name="wstage", bufs=ws_bufs))
        wrpool = ctx.enter_context(tc.tile_pool(name="wrpool", bufs=wr_bufs))
        otp = ctx.enter_context(tc.tile_pool(name="otp", bufs=ot_bufs))
        xtbp = ctx.enter_context(tc.tile_pool(name="xtbp", bufs=1))
        ps_main = ctx.enter_context(
            tc.tile_pool(name="ps_main", bufs=4, space="PSUM"))
        ps_p1t = ctx.enter_context(
            tc.tile_pool(name="ps_p1t", bufs=1, space="PSUM"))
        ps_aux = ctx.enter_context(
            tc.tile_pool(name="ps_aux", bufs=3, space="PSUM"))

        # identity for PE transpose (f32 path)
        ident = const.tile([P, P], f32)
        make_identity(nc, ident)
        # PE p-state warm-up: the tensor engine reaches full clock only
        # after ~3us of continuous execution. Burn the startup DMA-latency
        # window on dummy transposes so real work starts at full speed.
        for _ in range(12):
            wup = ps_aux.tile([P, P], f32, name="mm")
            nc.tensor.transpose(wup[:], ident[:], ident[:])

        # lora_A as [P, KT, r] f32r (one DMA; partition p holds rows ki*P+p)
        a_st = stg.tile([P, KT, r], f32, name="stg")
        a_r = const.tile([P, KT, r], f32r)
        # (a_r rounding copy is emitted inside the first k-loop)

        b17 = const.tile([R17, d], f32r)
        p1t = const.tile([R17, m_core], f32r)

        def stage_consts(step):
            # Emitted mid-k-loop of the first half-pass: keeps these DVE
            # copies out of the startup critical path.
            if step < 4:
                # b17 = [lora_B; bias]: [R17, d] f32r, staged in quarters
                qd = d // 4
                hsl = slice(step * qd, (step + 1) * qd)
                bst = stg.tile([R17, qd], f32, name="stg")
                nc.scalar.dma_start(bst[0:r, :], lora_B[:, hsl])
                nc.scalar.dma_start(bst[r:R17, :], bias[hsl].unsqueeze(0))
                nc.vector.tensor_copy(b17[:, hsl], bst[:])
            else:
                # p1t pre-fill with rounded 1.0: row 16 keeps it (ones for
                # the bias outer product); rows 0..16 get overwritten by
                # the per-half-pass chunk copies. Staged in halves.
                hm = m_core // 2
                hsl = slice((step - 4) * hm, (step - 3) * hm)
                p1st = stg.tile([R17, hm], f32, name="stg")
                nc.gpsimd.memset(p1st[:], 1.0)
                nc.vector.tensor_copy(p1t[:, hsl], p1st[:])

        # W tiles are emitted one consumer-step ahead (DMA + DVE rounding
        # copy), so at n-tile/block boundaries the next wr is already
        # rounded before the PSUM drain copies queue up on DVE.
        wsteps = []
        for mb in m_blocks:
            for hp in range(mb // 4):
                wsteps += [(0, ki) for ki in range(KT)]
            for ni in range(1, NT):
                wsteps += [(ni, ki) for ki in range(KT)]
        wq = []
        widx = [0]

        def emit_next_w():
            if widx[0] < len(wsteps):
                ni, ki = wsteps[widx[0]]
                widx[0] += 1
                nsl = slice(ni * n_tile, (ni + 1) * n_tile)
                ws = wstage.tile([P, n_tile], f32, name="ws")
                nc.sync.dma_start(ws[:], W[ki * P:(ki + 1) * P, nsl])
                wr = wrpool.tile([P, n_tile], f32r, name="wr")
                nc.vector.tensor_copy(wr[:], ws[:])
                wq.append(wr)

        emit_next_w()

        out_dmas = []
        pend = [None]

        def stage_out(ps_tile, mt, nsl_):
            # PSUM -> SBUF, paired: two adjacent m-tiles share one
            # staging tile and one DRAM store (halves the Act SEQ
            # DMA-prep serialization at boundaries). First half on
            # DVE, second on Act so the drain burst fits both queues.
            if pend[0] is None:
                ot2 = otp.tile([P, 2, n_tile], f32, name="ot")
                pend[0] = (ot2, mt)
                nc.vector.tensor_copy(ot2[:, 0, :], ps_tile[:])
            else:
                ot2, lo = pend[0]
                pend[0] = None
                assert mt == lo + 1
                nc.scalar.copy(ot2[:, 1, :], ps_tile[:])
                while out_dmas:
                    emit_out_dma(*out_dmas.pop(0))
                out_dmas.append((lo, nsl_, ot2))

        odma_flip = [0]

        def emit_out_dma(lo, nsl_, ot2):
            # alternate queues so back-to-back stores transfer in parallel
            eng = nc.scalar if odma_flip[0] % 2 == 0 else nc.sync
            odma_flip[0] += 1
            eng.dma_start(
                out[lo * P:(lo + 2) * P, nsl_].rearrange(
                    "(j p) f -> p j f", p=P), ot2[:])

        def flush_out_dmas():
            while out_dmas:
                emit_out_dma(*out_dmas.pop(0))

        mt0 = 0
        first = True
        for mb in m_blocks:
            xtb = xtbp.tile([P, KT, mb * P], f32r, name="xtb")
            nhp = mb // 4
            for hp in range(nhp):
                # ---- fused first n-tile pass (ni=0): transpose + GEMM
                mis = [hp * 4 + i for i in range(4)]
                nsl = slice(0, n_tile)
                pss = [ps_main.tile([P, n_tile], f32, name="mm") for _ in mis]
                pp1 = ps_p1t.tile([r, n_tile], f32, name="mm")
                xs_tiles = {}

                def load_group(g, split=False):
                    for i, mi in enumerate(mis):
                        mt = mt0 + mi
                        xst = xsp.tile([P, kg * P], f32, name="xs")
                        # the very first group gates kernel startup: Pool's
                        # SWDGE descriptor gen is ~1us per DMA serial, so
                        # split it across the Pool and Act queues
                        eng = nc.scalar if split and i % 2 == 0 else nc.gpsimd
                        eng.dma_start(
                            xst[:],
                            x[mt * P:(mt + 1) * P, g * kg * P:(g + 1) * kg * P])
                        xs_tiles[(mi, g)] = xst

                def transpose_ki(ki):
                    g, lk = ki // kg, ki % kg
                    for mi in mis:
                        pst = ps_aux.tile([P, P], f32, name="mm")
                        nc.tensor.transpose(
                            pst[:],
                            xs_tiles[(mi, g)][:, lk * P:(lk + 1) * P],
                            ident[:])
                        # copyback rounds to f32r for the main matmuls
                        nc.vector.tensor_copy(
                            xtb[:, ki, mi * P:(mi + 1) * P], pst[:])

                def drain(j, mi):
                    # lora+bias accumulation, then drain
                    mt = mt0 + mi
                    nc.tensor.matmul(
                        pss[j][:], p1t[:, mt * P:(mt + 1) * P], b17[:, nsl],
                        start=False, stop=True)
                    stage_out(pss[j], mt, nsl)

                def pp1_mm(ki):
                    nc.tensor.matmul(
                        pp1[:], a_r[:, ki, :],
                        xtb[:, ki, hp * n_tile:(hp + 1) * n_tile],
                        start=(ki == 0), stop=(ki == KT - 1))

                load_group(0, split=first)
                if first:
                    # a_st DMA after the startup-critical first x chunks
                    nc.scalar.dma_start(
                        a_st[:], lora_A.rearrange("(k p) r -> p k r", p=P))
                transpose_ki(0)
                for ki in range(KT):
                    wr = wq.pop(0)
                    emit_next_w()
                    if ki % kg == 0 and ki + kg < KT:
                        load_group(ki // kg + 1)
                    if ki + 1 < KT:
                        transpose_ki(ki + 1)
                    last = ki == KT - 1
                    for j, mi in enumerate(mis):
                        nc.tensor.matmul(
                            pss[j][:], xtb[:, ki, mi * P:(mi + 1) * P],
                            wr[:], start=(ki == 0), stop=False)
                        if last:
                            drain(j, mi)
                    if first and ki == 0:
                        # a_r rounding copy off the startup critical path
                        # (first needed by pp1_mm(0) below)
                        nc.vector.tensor_copy(a_r[:], a_st[:])
                    # pp1 runs one k-tile behind the transposes but one AHEAD
                    # of this loop for ki>=1, so its stop lands in iteration
                    # KT-2 and the p1t rounding copy hides under the last
                    # main matmuls.
                    if ki == 0:
                        pp1_mm(0)
                        pp1_mm(1)
                    elif ki < KT - 1:
                        pp1_mm(ki + 1)
                        if ki == KT - 2:
                            nc.vector.tensor_copy(
                                p1t[0:r, (mt0 + hp * 4) * P:
                                    (mt0 + hp * 4) * P + n_tile],
                                pp1[:])
                    if first and 4 <= ki < 28 and ki % 4 == 0:
                        stage_consts(ki // 4 - 1)
                flush_out_dmas()
                first = False

            # ---- remaining n-tiles: plain GEMM with 8-wide PSUM
            for ni in range(1, NT):
                nsl = slice(ni * n_tile, (ni + 1) * n_tile)
                pss = []
                for mi in range(mb):
                    pool = (ps_main if mi < 4 else
                            ps_aux if mi < 7 else ps_p1t)
                    pss.append(pool.tile([P, n_tile], f32, name="mm"))
                # at the last n-tile of a block, drain the aux/p1t banks
                # first: the next block's transposes/pp1 need those slots
                # before the main banks
                mi_order = (list(range(4, mb)) + list(range(4))
                            if ni == NT - 1 else list(range(mb)))
                for ki in range(KT):
                    wr = wq.pop(0)
                    emit_next_w()
                    last = ki == KT - 1
                    for mi in (mi_order if last else range(mb)):
                        mt = mt0 + mi
                        nc.tensor.matmul(
                            pss[mi][:], xtb[:, ki, mi * P:(mi + 1) * P],
                            wr[:], start=(ki == 0), stop=False)
                        if last:
                            nc.tensor.matmul(
                                pss[mi][:], p1t[:, mt * P:(mt + 1) * P],
                                b17[:, nsl], start=False, stop=True)
                            stage_out(pss[mi], mt, nsl)
                flush_out_dmas()
            mt0 += mb
    nc.compile()
    return nc


_CACHE = {}


def _get_nc(key, *args, **kw):
    if key not in _CACHE:
        _CACHE[key] = build_nc(*args, **kw)
    return _CACHE[key]


def kernel(x, W, bias, lora_A, lora_B, _trace=False):
    Bb, S, D = x.shape
    R = lora_A.shape[1]
    M = Bb * S
    m_core = M // NCORES
    m_blocks = [8, 8]
    nc = _get_nc(("full", m_core, D, R), m_core, D, R, m_blocks)

    xf = np.ascontiguousarray(x.reshape(M, D), dtype=np.float32)
    W = np.ascontiguousarray(W, dtype=np.float32)
    bias = np.ascontiguousarray(bias, dtype=np.float32)
    lora_A = np.ascontiguousarray(lora_A, dtype=np.float32)
    lora_B = np.ascontiguousarray(lora_B, dtype=np.float32)

    in_maps = []
    for c in range(NCORES):
        in_maps.append({
            "x": xf[c * m_core:(c + 1) * m_core],
            "W": W, "bias": bias, "lora_A": lora_A, "lora_B": lora_B,
        })
    res = run_bass_kernel_spmd(nc, in_maps, list(range(NCORES)), trace=_trace)
    outs = [res.results[c]["out"] for c in range(NCORES)]
    full = np.concatenate(outs, axis=0).reshape(Bb, S, D).astype(x.dtype)
    if _trace:
        return full, res
    return full
